# revision 35
# baseline (speedup 1.0000x reference)
"""Multi-head attention with relative position bias on 8 trn2 NeuronCores.

Sharding: data-parallel on batch (2) x tensor-parallel on heads (16 -> 4 per
core).  Core c handles batch c//4, heads 4*(c%4) .. 4*(c%4)+3.  Each core
computes its 4 heads' attention and a partial output projection; the host sums
the 4 partials per batch and adds b_out.

Device-side design (per core), all bf16 on the PE (no fp8: e4m3's subnormal
floor costs ~2.3% rel err on softmax weights, over the 2e-2 gate):
  - x^T, W_q/W_k/W_v^T, W_out^T host-cast to bf16 (halves the startup DMA
    vs fp32r at the same 1 col/cycle PE rate; rel err 0.0079 in numpy sim).
  - QKV projections run dt-outer so one LdWeights serves the 4 matmuls of
    all query chunks (InstMatmult.ldweights=False on the followers).
  - scores computed transposed: S^T[k, q] = kT-tile @ qT, contraction dh=64
    on partitions; per k-tile ONE ldweights feeds both 512-col halves of a
    1024-wide query unit.  1/8 scale folded into W_k on the host.
  - rel-pos bias applied MULTIPLICATIVELY after exp: pt = exp(S) * exp(b).
    Act reads PSUM directly (no DVE pre-pass), writes bf16; DVE (14/16
    k-tiles) and Pool (2/16 -- it runs at 0.42 roofline) then do ONE
    full-width multiply per k-tile against a host-precomputed 33-tile
    exp(bias) table per head ([8 lo-edge | 17 mid Toeplitz | 8 hi-edge],
    so the window base 16-kt+8u is always a contiguous slice and edge
    saturation needs no separate pass).  bf16xbf16 multiplies run the
    DVE 2x path and the multiplicative form skips the baseline's bf16
    rounding of S+b (which cost ~1.6% on large scores).
  - P@V per unit: 16 k-tiles x 2 accumulating bf16 matmuls (512-col PSUM
    bank limit), lhsT = v-tile [128, 65] (ones column -> softmax
    denominator l in psum row 64).
  - softmax reciprocal BATCHED per unit: the l row bounces PSUM->SBUF->
    DRAM, is gathered back as [128, 8] so ln/exp on Act use all 128
    lanes (vs 1), and returns via DRAM for the partition-broadcast
    multiply.  Every DMA in the chain sits on the gpsimd queue: DRAM
    read-after-write is NOT dependency-tracked, only same-queue FIFO
    order protects it.
  - output projection with heads PAIRED to contraction 128: the normalize
    multiply writes odd heads partition-shifted into rows 64:128 of a
    [128, 2, n] attn tile (DVE supports shifted writes; HW-verified), so
    out-proj needs 2 accumulation steps instead of 4.
  - emission interleaves PE work at k-tile granularity so the PE never
    stalls on the 2-deep score-psum rotation while Act drains it: the
    first score unit hosts the v-projection chains + ep1 q/k projection
    passes, every later unit the previous unit's P@V matmuls (keeps pt
    alive-generations at 2 = pool bufs), and the final P@V unit hosts
    the first half of the output projection.  PSUM: 4 tags x 4KB
    (s0,s1 scores+out-proj / pv0,pv1 P@V+projections) = all 16KB.
  - bass lowering emits one InstLdweights per matmul; _dedupe_ldweights
    post-pass replaces loads whose weights AP matches the previous load
    with sync-carrying NoOps (the PE array is weight-stationary), cutting
    896 -> ~465 weight loads (~35us of serialized PE time on HW).
  - _fix_sync_waits post-pass (unchanged from baseline) elides redundant
    semaphore waits and spills over-capacity wait lists onto NoOp carriers
    for walrus's per-struct sync-wait limits.
"""

import sys

import numpy as np

if "/opt/trn_rl_repo" not in sys.path:
    sys.path.insert(0, "/opt/trn_rl_repo")

import ml_dtypes

import concourse.bass as bass
import concourse.mybir as mybir
import concourse.tile as tile
from concourse.bass_utils import run_bass_kernel_spmd

F32 = mybir.dt.float32
BF16 = mybir.dt.bfloat16
EXP = mybir.ActivationFunctionType.Exp
LN = mybir.ActivationFunctionType.Ln

N = 2048  # sequence length
DIM = 1024  # model dim
HL = 4  # local heads per core
DH = 64  # head dim
NKT = N // 128  # 16 key tiles
CW = 1024  # query-unit width (2 units per head)
NDT = DIM // 128  # 8 contraction tiles for the projections

_PROGRAM = None
LAST_RESULTS = None  # BassKernelResults of the most recent run (for test.py)


def _emit(tc, xT, wqT, wkT, wvT, wo2T, expbT, l_dram, rec_dram, out_p):
    nc = tc.nc

    with (
        tc.tile_pool(name="persist", bufs=1) as persist,
        tc.tile_pool(name="pt", bufs=2) as ptp,
        tc.tile_pool(name="small", bufs=1) as smp,
        tc.tile_pool(name="ostp", bufs=2) as ostp,
        tc.tile_pool(name="ps", bufs=1, space="PSUM") as ps,
    ):
        # ---- constants + persistent tensors -------------------------------
        attn2_sb = persist.tile([128, 2, N], BF16)  # paired heads, normalized
        wo2_sb = persist.tile([128, 2, DIM], BF16)
        # 33 tiles per head: [8 lo-edge const | 17 mid Toeplitz | 8 hi-edge]
        # so every (kt, unit) bias multiply is ONE contiguous [128,1024] slice
        expb_sb = persist.tile([128, HL, 33, 128], BF16)

        q_sb = persist.tile([128, 2, N], BF16)  # [2 heads x dh, ep, n]
        k_sb = persist.tile([128, 2, N], BF16)
        v_sb = persist.tile([128, NKT, HL, DH + 1], BF16)  # + ones column
        nc.vector.memset(v_sb[:, :, :, DH : DH + 1], 1.0)

        x_sb = persist.tile([128, NDT, N], BF16)
        wq_sb = persist.tile([128, NDT, 256], BF16)
        wk_sb = persist.tile([128, NDT, 256], BF16)
        wv_sb = persist.tile([128, NDT, 256], BF16)
        # weights + bias tables on the scalar hwdge queue; x split across
        # the sync and gpsimd queues dt-major so the dt-outer projections
        # can start as soon as the first dim-tiles land
        # wk rides FIRST on the sync queue (it gates the very first matmul
        # and the sync hwdge starts ~3us before the scalar one)
        nc.sync.dma_start(out=wk_sb, in_=wkT.rearrange("(t p) e -> p t e", p=128))
        nc.scalar.dma_start(out=wq_sb, in_=wqT.rearrange("(t p) e -> p t e", p=128))
        nc.scalar.dma_start(out=wv_sb, in_=wvT.rearrange("(t p) e -> p t e", p=128))
        for dt in range(NDT):
            eng = (nc.sync, nc.gpsimd, nc.sync, nc.gpsimd, nc.sync,
                   nc.gpsimd, nc.scalar, nc.scalar)[dt]
            eng.dma_start(
                out=x_sb[:, dt, :], in_=xT[dt * 128 : (dt + 1) * 128, :]
            )
        nc.scalar.dma_start(out=expb_sb, in_=expbT)
        nc.scalar.dma_start(out=wo2_sb, in_=wo2T)

        # ---- QKV projections (bf16, dt-outer, shared ldweights) -----------
        def proj4(wsb, osb, ep, tags):
            """All 4 query chunks of one 128-row output slab in a single
            dt-outer pass: 8 ldweights, 32 matmuls on two [128,1024] psums."""
            p0 = ps.tile([128, CW], F32, tag=tags[0])
            p1 = ps.tile([128, CW], F32, tag=tags[1])
            for dt in range(NDT):
                lhsT = wsb[:, dt, ep * 128 : (ep + 1) * 128]
                first = True
                for p, base in ((p0, 0), (p0, 512), (p1, 1024), (p1, 1536)):
                    mm = nc.tensor.matmul(
                        p[:, base % CW : base % CW + 512],
                        lhsT=lhsT,
                        rhs=x_sb[:, dt, base : base + 512],
                        start=(dt == 0),
                        stop=(dt == NDT - 1),
                    )
                    if not first:
                        mm.ins.ldweights = False
                    first = False
            nc.any.tensor_copy(osb[:, ep, 0:CW], p0)
            nc.any.tensor_copy(osb[:, ep, CW : 2 * CW], p1)

        def proj2(wsb, osb, ep, half, tag):
            """One chunk-pair of an ep1 slab on a single psum tag (used to
            interleave with score units without touching the s tags)."""
            p = ps.tile([128, CW], F32, tag=tag)
            for dt in range(NDT):
                lhsT = wsb[:, dt, ep * 128 : (ep + 1) * 128]
                mm0 = nc.tensor.matmul(
                    p[:, 0:512],
                    lhsT=lhsT,
                    rhs=x_sb[:, dt, half * CW : half * CW + 512],
                    start=(dt == 0),
                    stop=(dt == NDT - 1),
                )
                mm1 = nc.tensor.matmul(
                    p[:, 512:CW],
                    lhsT=lhsT,
                    rhs=x_sb[:, dt, half * CW + 512 : half * CW + CW],
                    start=(dt == 0),
                    stop=(dt == NDT - 1),
                )
                mm1.ins.ldweights = False
            nc.any.tensor_copy(osb[:, ep, half * CW : (half + 1) * CW], p)

        def v_chain(kt):
            p = ps.tile([128, 256], F32, tag=f"pv{kt % 2}")
            for dt in range(NDT):
                nc.tensor.matmul(
                    p,
                    lhsT=x_sb[:, dt, kt * 128 : (kt + 1) * 128],
                    rhs=wv_sb[:, dt, :],
                    start=(dt == 0),
                    stop=(dt == NDT - 1),
                )
            nc.any.tensor_copy(v_sb[:, kt, :, 0:DH], p)

        # ---- attention building blocks ------------------------------------
        pt_tiles = {}

        def sc_tile(h, u, kt):
            """One k-tile of unit (h, u): 2 score matmuls (shared ldweights),
            exp from psum, multiplicative bias."""
            hp, hr = divmod(h, 2)
            qrow = hr * 64
            pt = pt_tiles[(h, u)]
            p = ps.tile([128, CW], F32, tag=f"s{kt % 2}")
            lhsT = k_sb[qrow : qrow + 64, hp, kt * 128 : (kt + 1) * 128]
            for half in range(2):
                mm = nc.tensor.matmul(
                    p[:, half * 512 : (half + 1) * 512],
                    lhsT=lhsT,
                    rhs=q_sb[
                        qrow : qrow + 64, hp,
                        u * CW + half * 512 : u * CW + (half + 1) * 512,
                    ],
                    start=True,
                    stop=True,
                )
                if half == 1:
                    mm.ins.ldweights = False
            nc.scalar.activation(pt[:, kt, :], p, EXP)
            # one full-width bias multiply; table index base = 16 - kt + 8u
            # (Pool runs at 0.42 roofline: give it only 2 of 16 tiles)
            eng = nc.gpsimd if kt % 8 == 0 else nc.vector
            base = 16 - kt + 8 * u
            dst = pt[:, kt, :]
            eng.tensor_mul(dst, dst, expb_sb[:, h, base : base + 8, :])

        pv_tiles = {}

        def pv_mm(h, u, kt):
            pt = pt_tiles[(h, u)]
            if kt == 0:
                pv_tiles[(h, u)] = ps.tile([DH + 1, CW], F32, tag=f"pv{u}", name=f"pv_{h}_{u}")
            # matmul output must stay within one PSUM bank: 2x512 halves
            for half in range(2):
                nc.tensor.matmul(
                    pv_tiles[(h, u)][:, half * 512 : (half + 1) * 512],
                    lhsT=v_sb[:, kt, h, :],
                    rhs=pt[:, kt, half * 512 : (half + 1) * 512],
                    start=(kt == 0),
                    stop=(kt == NKT - 1),
                )
            if kt == NKT - 1:
                # DMA (and Pool) cannot read PSUM: bounce l through SBUF
                r = 2 * h + u
                l_sb = smp.tile([1, CW], F32, tag=f"l{u}", name=f"l_{h}_{u}")
                nc.vector.tensor_copy(l_sb, pv_tiles[(h, u)][DH : DH + 1, :])
                nc.gpsimd.dma_start(out=l_dram[r : r + 1, :], in_=l_sb)

        def rec_attn(h, u):
            """Batched 128-lane reciprocal for unit (h, u) + normalize-mul
            into the paired attn tile (odd heads partition-shifted)."""
            # NOTE: every DMA in this chain sits on the gpsimd queue (with
            # the l-row store in pv_mm): DRAM read-after-write is NOT
            # dependency-tracked, only same-queue FIFO order protects it
            r = 2 * h + u
            rec_sb = smp.tile([128, 8], F32, tag=f"rec{u}", name=f"rec_{h}_{u}")
            nc.gpsimd.dma_start(
                out=rec_sb,
                in_=bass.AP(
                    tensor=l_dram.tensor, offset=r * CW,
                    ap=[[8, 128], [1, 8]],
                ),
            )
            nc.scalar.activation(rec_sb, rec_sb, LN)
            nc.scalar.activation(rec_sb, rec_sb, EXP, scale=-1.0)
            nc.gpsimd.dma_start(
                out=bass.AP(
                    tensor=rec_dram.tensor, offset=r * CW,
                    ap=[[8, 128], [1, 8]],
                ),
                in_=rec_sb,
            )
            hp, hr = divmod(h, 2)
            rb = hr * 64
            bc = smp.tile([64, CW], F32, tag=f"bc{u}", name=f"bc_{h}_{u}")
            nc.gpsimd.dma_start(
                out=bc,
                in_=bass.AP(
                    tensor=rec_dram.tensor, offset=r * CW,
                    ap=[[0, 64], [1, CW]],
                ),
            )
            nc.vector.tensor_mul(
                attn2_sb[rb : rb + 64, hp, u * CW : (u + 1) * CW],
                pv_tiles[(h, u)][0:DH, :],
                bc,
            )

        # ---- emission: QKV + attention pipeline ---------------------------
        proj4(wk_sb, k_sb, 0, ("s0", "s1"))
        proj4(wq_sb, q_sb, 0, ("pv0", "pv1"))

        units = [(h, u) for h in range(HL) for u in range(2)]
        for i, (h, u) in enumerate(units):
            pt_tiles[(h, u)] = ptp.tile([128, NKT, CW], BF16, tag="pt", name=f"pt_{h}_{u}")
            for kt in range(NKT):
                sc_tile(h, u, kt)
                # interleave non-s-tag PE work so the PE is not capped by
                # the Act drain pace of the 2-tag score-psum rotation: the
                # first unit hosts the v-projection chains + ep1 q/k
                # projection passes, every later unit the previous unit's
                # P@V matmuls (keeps pt alive-generations at 2 = pool bufs)
                if i == 0:
                    v_chain(kt)
                    if kt % 4 == 3:
                        pr = kt // 4
                        proj2(
                            (wk_sb, wk_sb, wq_sb, wq_sb)[pr],
                            (k_sb, k_sb, q_sb, q_sb)[pr],
                            1, pr % 2, f"pv{pr % 2}",
                        )
                else:
                    ph, pu = units[i - 1]
                    pv_mm(ph, pu, kt)
                    if kt == NKT - 1:
                        rec_attn(ph, pu)

        # ---- output projection (paired heads, contraction 128) ------------
        def outproj(qi):
            # rotate over 3-4 psum tags (pv0 frees after attnmul(3,0), pv1
            # after attnmul(3,1)) so matmuls never wait on the psum drain,
            # and alternate the drain copies between Act and DVE
            tags = ("s0", "s1", "pv0") if qi < 8 else ("s0", "s1", "pv0", "pv1")
            po = ps.tile([128, DIM], F32, tag=tags[qi % len(tags)], name=f"po_{qi}")
            for hp in range(2):
                for half in range(2):
                    nc.tensor.matmul(
                        po[:, half * 512 : (half + 1) * 512],
                        lhsT=attn2_sb[:, hp, qi * 128 : (qi + 1) * 128],
                        rhs=wo2_sb[:, hp, half * 512 : (half + 1) * 512],
                        start=(hp == 0),
                        stop=(hp == 1),
                    )
            ost = ostp.tile([128, DIM], BF16, tag=f"ost{qi % 2}", name=f"ost_{qi}")
            if qi % 2:
                nc.vector.tensor_copy(ost, po)
            else:
                nc.any.tensor_copy(ost, po)
            # spread the 4MB of stores over all three DMA queues (disjoint
            # DRAM regions, no readers -> cross-queue order is safe)
            oq = (nc.sync, nc.scalar, nc.gpsimd)[qi % 3]
            oq.dma_start(out=out_p[qi * 128 : (qi + 1) * 128, :], in_=ost)

        # tail: ALL of the last P@V unit first, then outproj qi<8 EMITTED
        # BEFORE rec_attn(3,1) -- attn-tile dependency tracking is
        # conservative, so anything emitted after the final attnmul waits
        # on it even for disjoint columns.  The reciprocal chain then runs
        # on DMA/Act/DVE underneath the qi<8 matmuls.
        for kt in range(NKT):
            pv_mm(3, 1, kt)
        for qi in range(8):
            outproj(qi)
        rec_attn(3, 1)
        for qi in range(8, N // 128):
            outproj(qi)


def _dedupe_ldweights(nc):
    """Drop InstLdweights whose weights AP + modes are identical to the
    previous InstLdweights in the stream (the PE array is weight-stationary;
    only another ldweights clobbers it -- no transposes in this kernel).
    Each dropped load becomes a NoOp carrying its sync waits/updates, so
    dependency semantics are unchanged; _fix_sync_waits (run after) spills
    any over-capacity wait lists."""
    for f in nc.m.functions:
        for b in f.blocks:
            prev_key = None
            out = []
            for i in b.instructions:
                if isinstance(i, mybir.InstLdweights):
                    key = (
                        str(i.ins[0]),
                        str(i.perf_mode),
                        str(i.is_transpose),
                        str(i.tile_position),
                        str(i.tile_size),
                    )
                    if key == prev_key:
                        nop = mybir.InstNoOp(
                            name=nc.get_next_instruction_name(), ins=[], outs=[]
                        )
                        nop.engine = i.engine
                        nop.sync_info = i.sync_info
                        nop.bass_nofuse = True
                        out.append(nop)
                        continue
                    prev_key = key
                out.append(i)
            b.instructions = out


def _fix_sync_waits(nc):
    """Post-schedule wait hygiene for walrus's per-struct sync-wait limits.

    1. Elide waits already implied by an earlier wait on the same engine
       (sem-ge is monotone and engines execute their instructions in order).
    2. For instructions still over their struct's wait capacity, INSERT
       NoOp wait-carriers on the same engine directly before them (strictly
       more conservative: the waits execute earlier in the same engine
       order).
    """
    import re

    _elidable = re.compile(r"^(DMASW|DMAHW|PE|DVE|Activation|Pool|SP)")
    # only instruction types whose sync_info round-trips cleanly may be
    # touched; anything else (raw-ISA customs, barriers, drains, branches)
    # is left intact and clears the elision state conservatively
    _touchable = (
        mybir.InstMatmult,
        mybir.InstNoOp,
        mybir.InstTensorTensor,
        mybir.InstTensorScalarPtr,
        mybir.InstActivation,
        mybir.InstTensorCopy,
        mybir.InstDMACopy,
        mybir.InstLdweights,
        mybir.InstMemset,
    )
    for f in nc.m.functions:
        for b in f.blocks:
            seen = {}
            for i in b.instructions:
                si = i.sync_info
                if si is None or not si.on_wait:
                    continue
                if not isinstance(i, _touchable):
                    seen.clear()
                    continue
                s = seen.setdefault(i.engine, {})
                kept = []
                for w in si.on_wait:
                    if (
                        w.wait_mode == "sem-ge-imm"
                        and _elidable.match(w.ant_name or "")
                        and s.get(w.id, -1) >= w.wait_value
                    ):
                        continue
                    kept.append(w)
                    if w.wait_mode == "sem-ge-imm" and _elidable.match(
                        w.ant_name or ""
                    ):
                        s[w.id] = w.wait_value
                if len(kept) != len(si.on_wait):
                    si.on_wait = kept

    # capacity per opcode (walrus setupSyncWait limits, found empirically:
    # Matmult fp32r=1, NoOp=1; others conservative)
    def cap_of(i):
        if isinstance(i, mybir.InstDrain):
            return 1  # spill the kernel-tail drain's wait pile onto NoOps
        if not isinstance(i, _touchable):
            return None
        return 1

    for f in nc.m.functions:
        for b in f.blocks:
            out = []
            for i in b.instructions:
                si = i.sync_info
                cap = cap_of(i)
                if si is not None and si.on_wait and cap is not None and len(
                    si.on_wait
                ) > cap:
                    waits = list(si.on_wait)
                    excess, keep = waits[:-cap], waits[-cap:]
                    while excess:
                        chunk, excess = excess[:1], excess[1:]
                        nop = mybir.InstNoOp(
                            name=nc.get_next_instruction_name(), ins=[], outs=[]
                        )
                        nop.engine = i.engine
                        nop.sync_info = mybir.SyncInfo(on_wait=chunk, on_update=[])
                        nop.bass_nofuse = True
                        out.append(nop)
                    si.on_wait = keep
                out.append(i)
            b.instructions = out


def build_program():
    global _PROGRAM
    if _PROGRAM is not None:
        return _PROGRAM
    nc = bass.Bass(trn_type="TRN2", target_bir_lowering=False, debug=False)
    xT = nc.dram_tensor("xT", [DIM, N], BF16, kind="ExternalInput").ap()
    wqT = nc.dram_tensor("wqT", [DIM, 256], BF16, kind="ExternalInput").ap()
    wkT = nc.dram_tensor("wkT", [DIM, 256], BF16, kind="ExternalInput").ap()
    wvT = nc.dram_tensor("wvT", [DIM, 256], BF16, kind="ExternalInput").ap()
    wo2T = nc.dram_tensor("wo2T", [128, 2, DIM], BF16, kind="ExternalInput").ap()
    expbT = nc.dram_tensor("expbT", [128, HL, 33, 128], BF16, kind="ExternalInput").ap()
    l_dram = nc.dram_tensor("l_scratch", [HL * 2, CW], F32, kind="Internal").ap()
    rec_dram = nc.dram_tensor("rec_scratch", [HL * 2, CW], F32, kind="Internal").ap()
    out_p = nc.dram_tensor("out_p", [N, DIM], BF16, kind="ExternalOutput").ap()

    with tile.TileContext(nc) as tc:
        _emit(tc, xT, wqT, wkT, wvT, wo2T, expbT, l_dram, rec_dram, out_p)
    _dedupe_ldweights(nc)
    _fix_sync_waits(nc)
    _PROGRAM = nc
    return nc


def make_in_maps(x, W_qkv, W_out, rel_emb):
    x = np.asarray(x, np.float32)
    W_qkv = np.asarray(W_qkv, np.float32)
    W_out = np.asarray(W_out, np.float32)
    rel_emb = np.asarray(rel_emb, np.float32)
    BF = ml_dtypes.bfloat16

    dd = np.arange(128)[:, None] - np.arange(128)[None, :]
    xTs = [np.ascontiguousarray(x[b].T).astype(BF) for b in range(x.shape[0])]
    woT = W_out.T  # [d, e]
    in_maps = []
    for c in range(8):
        b, g = c // 4, c % 4
        wq = W_qkv[g * 256 : (g + 1) * 256]
        wk = W_qkv[DIM + g * 256 : DIM + (g + 1) * 256] * np.float32(0.125)
        wv = W_qkv[2 * DIM + g * 256 : 2 * DIM + (g + 1) * 256]
        wo2 = np.ascontiguousarray(
            woT[256 * g : 256 * (g + 1)].reshape(2, 128, DIM).transpose(1, 0, 2)
        )
        # 33 Toeplitz tiles per head: e=8..24 are the mid window (block
        # delta kt-qi = 16-e), e<8 / e>24 saturate fully via the clip
        bT = np.empty((HL, 33, 128, 128), np.float32)
        for hl in range(HL):
            head = 4 * g + hl
            for e in range(33):
                i = e - 8
                idx = np.clip(128 * (8 - i) + dd, -1024, 1024) + 1024
                bT[hl, e] = np.exp(rel_emb[idx, head])
        in_maps.append(
            {
                "xT": xTs[b],
                "wqT": np.ascontiguousarray(wq.T).astype(BF),
                "wkT": np.ascontiguousarray(wk.T).astype(BF),
                "wvT": np.ascontiguousarray(wv.T).astype(BF),
                "wo2T": wo2.astype(BF),
                "expbT": np.ascontiguousarray(
                    bT.transpose(2, 0, 1, 3)
                ).astype(BF),
            }
        )
    return in_maps


def combine_outputs(results, b_out):
    b_out = np.asarray(b_out, np.float32)
    out = np.empty((2, N, DIM), np.float32)
    for b in range(2):
        acc = results[4 * b]["out_p"].astype(np.float32)
        for g in range(1, 4):
            acc = acc + results[4 * b + g]["out_p"].astype(np.float32)
        out[b] = acc + b_out[None, :]
    return out


def kernel(x, W_qkv, W_out, b_out, rel_emb):
    global LAST_RESULTS
    nc = build_program()
    in_maps = make_in_maps(x, W_qkv, W_out, rel_emb)
    LAST_RESULTS = run_bass_kernel_spmd(nc, in_maps, list(range(8)))
    return combine_outputs(LAST_RESULTS.results, b_out)


# revision 36
# speedup vs baseline: 1.0071x; 1.0071x over previous
"""Multi-head attention with relative position bias on 8 trn2 NeuronCores.

Sharding: data-parallel on batch (2) x tensor-parallel on heads (16 -> 4 per
core).  Core c handles batch c//4, heads 4*(c%4) .. 4*(c%4)+3.  Each core
computes its 4 heads' attention and a partial output projection; the host sums
the 4 partials per batch and adds b_out.

Device-side design (per core), all bf16 on the PE (no fp8: e4m3's subnormal
floor costs ~2.3% rel err on softmax weights, over the 2e-2 gate):
  - x^T, W_q/W_k/W_v^T, W_out^T host-cast to bf16 (halves the startup DMA
    vs fp32r at the same 1 col/cycle PE rate; rel err 0.0079 in numpy sim).
  - QKV projections run dt-outer so one LdWeights serves the 4 matmuls of
    all query chunks (InstMatmult.ldweights=False on the followers).
  - scores computed transposed: S^T[k, q] = kT-tile @ qT, contraction dh=64
    on partitions; per k-tile ONE ldweights feeds both 512-col halves of a
    1024-wide query unit.  1/8 scale folded into W_k on the host.
  - rel-pos bias applied MULTIPLICATIVELY after exp: pt = exp(S) * exp(b).
    Act reads PSUM directly (no DVE pre-pass), writes bf16; DVE (14/16
    k-tiles) and Pool (2/16 -- it runs at 0.42 roofline) then do ONE
    full-width multiply per k-tile against a host-precomputed 33-tile
    exp(bias) table per head ([8 lo-edge | 17 mid Toeplitz | 8 hi-edge],
    so the window base 16-kt+8u is always a contiguous slice and edge
    saturation needs no separate pass).  bf16xbf16 multiplies run the
    DVE 2x path and the multiplicative form skips the baseline's bf16
    rounding of S+b (which cost ~1.6% on large scores).
  - P@V per unit: 16 k-tiles x 2 accumulating bf16 matmuls (512-col PSUM
    bank limit), lhsT = v-tile [128, 65] (ones column -> softmax
    denominator l in psum row 64).
  - softmax reciprocal BATCHED per unit: the l row bounces PSUM->SBUF->
    DRAM, is gathered back as [128, 8] so ln/exp on Act use all 128
    lanes (vs 1), and returns via DRAM for the partition-broadcast
    multiply.  Every DMA in the chain sits on the gpsimd queue: DRAM
    read-after-write is NOT dependency-tracked, only same-queue FIFO
    order protects it.
  - output projection with heads PAIRED to contraction 128: the normalize
    multiply writes odd heads partition-shifted into rows 64:128 of a
    [128, 2, n] attn tile (DVE supports shifted writes; HW-verified), so
    out-proj needs 2 accumulation steps instead of 4.
  - emission interleaves PE work at k-tile granularity so the PE never
    stalls on the 2-deep score-psum rotation while Act drains it: the
    first score unit hosts the v-projection chains + ep1 q/k projection
    passes, every later unit the previous unit's P@V matmuls (keeps pt
    alive-generations at 2 = pool bufs), and the final P@V unit hosts
    the first half of the output projection.  PSUM: 4 tags x 4KB
    (s0,s1 scores+out-proj / pv0,pv1 P@V+projections) = all 16KB.
  - bass lowering emits one InstLdweights per matmul; _dedupe_ldweights
    post-pass replaces loads whose weights AP matches the previous load
    with sync-carrying NoOps (the PE array is weight-stationary), cutting
    896 -> ~465 weight loads (~35us of serialized PE time on HW).
  - _fix_sync_waits post-pass (unchanged from baseline) elides redundant
    semaphore waits and spills over-capacity wait lists onto NoOp carriers
    for walrus's per-struct sync-wait limits.
"""

import sys

import numpy as np

if "/opt/trn_rl_repo" not in sys.path:
    sys.path.insert(0, "/opt/trn_rl_repo")

import ml_dtypes

import concourse.bass as bass
import concourse.mybir as mybir
import concourse.tile as tile
from concourse.bass_utils import run_bass_kernel_spmd

F32 = mybir.dt.float32
BF16 = mybir.dt.bfloat16
EXP = mybir.ActivationFunctionType.Exp
LN = mybir.ActivationFunctionType.Ln

N = 2048  # sequence length
DIM = 1024  # model dim
HL = 4  # local heads per core
DH = 64  # head dim
NKT = N // 128  # 16 key tiles
CW = 1024  # query-unit width (2 units per head)
NDT = DIM // 128  # 8 contraction tiles for the projections

_PROGRAM = None
LAST_RESULTS = None  # BassKernelResults of the most recent run (for test.py)


def _emit(tc, xT, wqT, wkT, wvT, wo2T, expbT, l_dram, rec_dram, out_p):
    nc = tc.nc

    with (
        tc.tile_pool(name="persist", bufs=1) as persist,
        tc.tile_pool(name="pt", bufs=2) as ptp,
        tc.tile_pool(name="small", bufs=1) as smp,
        tc.tile_pool(name="ostp", bufs=2) as ostp,
        tc.tile_pool(name="ps", bufs=1, space="PSUM") as ps,
    ):
        # ---- constants + persistent tensors -------------------------------
        attn2_sb = persist.tile([128, 2, N], BF16)  # paired heads, normalized
        wo2_sb = persist.tile([128, 2, DIM], BF16)
        # 33 tiles per head: [8 lo-edge const | 17 mid Toeplitz | 8 hi-edge]
        # so every (kt, unit) bias multiply is ONE contiguous [128,1024] slice
        expb_sb = persist.tile([128, HL, 33, 128], BF16)

        q_sb = persist.tile([128, 2, N], BF16)  # [2 heads x dh, ep, n]
        k_sb = persist.tile([128, 2, N], BF16)
        v_sb = persist.tile([128, NKT, HL, DH + 1], BF16)  # + ones column
        nc.vector.memset(v_sb[:, :, :, DH : DH + 1], 1.0)

        x_sb = persist.tile([128, NDT, N], BF16)
        wq_sb = persist.tile([128, NDT, 256], BF16)
        wk_sb = persist.tile([128, NDT, 256], BF16)
        wv_sb = persist.tile([128, NDT, 256], BF16)
        # weights + bias tables on the scalar hwdge queue; x split across
        # the sync and gpsimd queues dt-major so the dt-outer projections
        # can start as soon as the first dim-tiles land
        # wk rides FIRST on the sync queue (it gates the very first matmul
        # and the sync hwdge starts ~3us before the scalar one)
        nc.sync.dma_start(out=wk_sb, in_=wkT.rearrange("(t p) e -> p t e", p=128))
        nc.scalar.dma_start(out=wq_sb, in_=wqT.rearrange("(t p) e -> p t e", p=128))
        nc.scalar.dma_start(out=wv_sb, in_=wvT.rearrange("(t p) e -> p t e", p=128))
        for dt in range(NDT):
            eng = (nc.sync, nc.gpsimd, nc.sync, nc.gpsimd, nc.sync,
                   nc.gpsimd, nc.scalar, nc.scalar)[dt]
            eng.dma_start(
                out=x_sb[:, dt, :], in_=xT[dt * 128 : (dt + 1) * 128, :]
            )
        nc.scalar.dma_start(out=expb_sb, in_=expbT)
        nc.scalar.dma_start(out=wo2_sb, in_=wo2T)

        # ---- QKV projections (bf16, dt-outer, shared ldweights) -----------
        def proj4(wsb, osb, ep, tags):
            """All 4 query chunks of one 128-row output slab in a single
            dt-outer pass: 8 ldweights, 32 matmuls on two [128,1024] psums."""
            p0 = ps.tile([128, CW], F32, tag=tags[0])
            p1 = ps.tile([128, CW], F32, tag=tags[1])
            for dt in range(NDT):
                lhsT = wsb[:, dt, ep * 128 : (ep + 1) * 128]
                first = True
                for p, base in ((p0, 0), (p0, 512), (p1, 1024), (p1, 1536)):
                    mm = nc.tensor.matmul(
                        p[:, base % CW : base % CW + 512],
                        lhsT=lhsT,
                        rhs=x_sb[:, dt, base : base + 512],
                        start=(dt == 0),
                        stop=(dt == NDT - 1),
                    )
                    if not first:
                        mm.ins.ldweights = False
                    first = False
            nc.vector.tensor_copy(osb[:, ep, 0:CW], p0)
            nc.vector.tensor_copy(osb[:, ep, CW : 2 * CW], p1)

        def proj2(wsb, osb, ep, half, tag):
            """One chunk-pair of an ep1 slab on a single psum tag (used to
            interleave with score units without touching the s tags)."""
            p = ps.tile([128, CW], F32, tag=tag)
            for dt in range(NDT):
                lhsT = wsb[:, dt, ep * 128 : (ep + 1) * 128]
                mm0 = nc.tensor.matmul(
                    p[:, 0:512],
                    lhsT=lhsT,
                    rhs=x_sb[:, dt, half * CW : half * CW + 512],
                    start=(dt == 0),
                    stop=(dt == NDT - 1),
                )
                mm1 = nc.tensor.matmul(
                    p[:, 512:CW],
                    lhsT=lhsT,
                    rhs=x_sb[:, dt, half * CW + 512 : half * CW + CW],
                    start=(dt == 0),
                    stop=(dt == NDT - 1),
                )
                mm1.ins.ldweights = False
            nc.vector.tensor_copy(osb[:, ep, half * CW : (half + 1) * CW], p)

        def v_chain(kt):
            p = ps.tile([128, 256], F32, tag=f"pv{kt % 2}")
            for dt in range(NDT):
                nc.tensor.matmul(
                    p,
                    lhsT=x_sb[:, dt, kt * 128 : (kt + 1) * 128],
                    rhs=wv_sb[:, dt, :],
                    start=(dt == 0),
                    stop=(dt == NDT - 1),
                )
            nc.vector.tensor_copy(v_sb[:, kt, :, 0:DH], p)

        # ---- attention building blocks ------------------------------------
        pt_tiles = {}

        def sc_tile(h, u, kt):
            """One k-tile of unit (h, u): 2 score matmuls (shared ldweights),
            exp from psum, multiplicative bias."""
            hp, hr = divmod(h, 2)
            qrow = hr * 64
            pt = pt_tiles[(h, u)]
            p = ps.tile([128, CW], F32, tag=f"s{kt % 2}")
            lhsT = k_sb[qrow : qrow + 64, hp, kt * 128 : (kt + 1) * 128]
            for half in range(2):
                mm = nc.tensor.matmul(
                    p[:, half * 512 : (half + 1) * 512],
                    lhsT=lhsT,
                    rhs=q_sb[
                        qrow : qrow + 64, hp,
                        u * CW + half * 512 : u * CW + (half + 1) * 512,
                    ],
                    start=True,
                    stop=True,
                )
                if half == 1:
                    mm.ins.ldweights = False
            nc.scalar.activation(pt[:, kt, :], p, EXP)
            # one full-width bias multiply; table index base = 16 - kt + 8u
            # (Pool runs at 0.42 roofline: give it only 2 of 16 tiles)
            eng = nc.gpsimd if kt % 8 == 0 else nc.vector
            base = 16 - kt + 8 * u
            dst = pt[:, kt, :]
            eng.tensor_mul(dst, dst, expb_sb[:, h, base : base + 8, :])

        pv_tiles = {}

        def pv_mm(h, u, kt):
            pt = pt_tiles[(h, u)]
            if kt == 0:
                pv_tiles[(h, u)] = ps.tile([DH + 1, CW], F32, tag=f"pv{u}", name=f"pv_{h}_{u}")
            # matmul output must stay within one PSUM bank: 2x512 halves
            for half in range(2):
                nc.tensor.matmul(
                    pv_tiles[(h, u)][:, half * 512 : (half + 1) * 512],
                    lhsT=v_sb[:, kt, h, :],
                    rhs=pt[:, kt, half * 512 : (half + 1) * 512],
                    start=(kt == 0),
                    stop=(kt == NKT - 1),
                )
            if kt == NKT - 1:
                # DMA (and Pool) cannot read PSUM: bounce l through SBUF
                r = 2 * h + u
                l_sb = smp.tile([1, CW], F32, tag=f"l{u}", name=f"l_{h}_{u}")
                nc.vector.tensor_copy(l_sb, pv_tiles[(h, u)][DH : DH + 1, :])
                nc.gpsimd.dma_start(out=l_dram[r : r + 1, :], in_=l_sb)

        def rec_attn(h, u):
            """Batched 128-lane reciprocal for unit (h, u) + normalize-mul
            into the paired attn tile (odd heads partition-shifted)."""
            # NOTE: every DMA in this chain sits on the gpsimd queue (with
            # the l-row store in pv_mm): DRAM read-after-write is NOT
            # dependency-tracked, only same-queue FIFO order protects it
            r = 2 * h + u
            rec_sb = smp.tile([128, 8], F32, tag=f"rec{u}", name=f"rec_{h}_{u}")
            nc.gpsimd.dma_start(
                out=rec_sb,
                in_=bass.AP(
                    tensor=l_dram.tensor, offset=r * CW,
                    ap=[[8, 128], [1, 8]],
                ),
            )
            nc.scalar.activation(rec_sb, rec_sb, LN)
            nc.scalar.activation(rec_sb, rec_sb, EXP, scale=-1.0)
            nc.gpsimd.dma_start(
                out=bass.AP(
                    tensor=rec_dram.tensor, offset=r * CW,
                    ap=[[8, 128], [1, 8]],
                ),
                in_=rec_sb,
            )
            hp, hr = divmod(h, 2)
            rb = hr * 64
            bc = smp.tile([64, CW], F32, tag=f"bc{u}", name=f"bc_{h}_{u}")
            nc.gpsimd.dma_start(
                out=bc,
                in_=bass.AP(
                    tensor=rec_dram.tensor, offset=r * CW,
                    ap=[[0, 64], [1, CW]],
                ),
            )
            nc.vector.tensor_mul(
                attn2_sb[rb : rb + 64, hp, u * CW : (u + 1) * CW],
                pv_tiles[(h, u)][0:DH, :],
                bc,
            )

        # ---- emission: QKV + attention pipeline ---------------------------
        proj4(wk_sb, k_sb, 0, ("s0", "s1"))
        proj4(wq_sb, q_sb, 0, ("pv0", "pv1"))

        units = [(h, u) for h in range(HL) for u in range(2)]
        for i, (h, u) in enumerate(units):
            pt_tiles[(h, u)] = ptp.tile([128, NKT, CW], BF16, tag="pt", name=f"pt_{h}_{u}")
            for kt in range(NKT):
                sc_tile(h, u, kt)
                # interleave non-s-tag PE work so the PE is not capped by
                # the Act drain pace of the 2-tag score-psum rotation: the
                # first unit hosts the v-projection chains + ep1 q/k
                # projection passes, every later unit the previous unit's
                # P@V matmuls (keeps pt alive-generations at 2 = pool bufs)
                if i == 0:
                    v_chain(kt)
                    if kt % 4 == 3:
                        pr = kt // 4
                        proj2(
                            (wk_sb, wk_sb, wq_sb, wq_sb)[pr],
                            (k_sb, k_sb, q_sb, q_sb)[pr],
                            1, pr % 2, f"pv{pr % 2}",
                        )
                else:
                    ph, pu = units[i - 1]
                    pv_mm(ph, pu, kt)
                    if kt == NKT - 1:
                        rec_attn(ph, pu)

        # ---- output projection (paired heads, contraction 128) ------------
        def outproj(qi):
            # rotate over 3-4 psum tags (pv0 frees after attnmul(3,0), pv1
            # after attnmul(3,1)) so matmuls never wait on the psum drain,
            # and alternate the drain copies between Act and DVE
            tags = ("s0", "s1", "pv0") if qi < 8 else ("s0", "s1", "pv0", "pv1")
            po = ps.tile([128, DIM], F32, tag=tags[qi % len(tags)], name=f"po_{qi}")
            for hp in range(2):
                for half in range(2):
                    nc.tensor.matmul(
                        po[:, half * 512 : (half + 1) * 512],
                        lhsT=attn2_sb[:, hp, qi * 128 : (qi + 1) * 128],
                        rhs=wo2_sb[:, hp, half * 512 : (half + 1) * 512],
                        start=(hp == 0),
                        stop=(hp == 1),
                    )
            ost = ostp.tile([128, DIM], BF16, tag=f"ost{qi % 2}", name=f"ost_{qi}")
            if qi % 2:
                nc.vector.tensor_copy(ost, po)
            else:
                nc.any.tensor_copy(ost, po)
            # spread the 4MB of stores over all three DMA queues (disjoint
            # DRAM regions, no readers -> cross-queue order is safe)
            oq = (nc.sync, nc.scalar, nc.gpsimd)[qi % 3]
            oq.dma_start(out=out_p[qi * 128 : (qi + 1) * 128, :], in_=ost)

        # tail: ALL of the last P@V unit first, then outproj qi<8 EMITTED
        # BEFORE rec_attn(3,1) -- attn-tile dependency tracking is
        # conservative, so anything emitted after the final attnmul waits
        # on it even for disjoint columns.  The reciprocal chain then runs
        # on DMA/Act/DVE underneath the qi<8 matmuls.
        for kt in range(NKT):
            pv_mm(3, 1, kt)
        for qi in range(8):
            outproj(qi)
        rec_attn(3, 1)
        for qi in range(8, N // 128):
            outproj(qi)


def _dedupe_ldweights(nc):
    """Drop InstLdweights whose weights AP + modes are identical to the
    previous InstLdweights in the stream (the PE array is weight-stationary;
    only another ldweights clobbers it -- no transposes in this kernel).
    Each dropped load becomes a NoOp carrying its sync waits/updates, so
    dependency semantics are unchanged; _fix_sync_waits (run after) spills
    any over-capacity wait lists."""
    for f in nc.m.functions:
        for b in f.blocks:
            prev_key = None
            out = []
            for i in b.instructions:
                if isinstance(i, mybir.InstLdweights):
                    key = (
                        str(i.ins[0]),
                        str(i.perf_mode),
                        str(i.is_transpose),
                        str(i.tile_position),
                        str(i.tile_size),
                    )
                    if key == prev_key:
                        nop = mybir.InstNoOp(
                            name=nc.get_next_instruction_name(), ins=[], outs=[]
                        )
                        nop.engine = i.engine
                        nop.sync_info = i.sync_info
                        nop.bass_nofuse = True
                        out.append(nop)
                        continue
                    prev_key = key
                out.append(i)
            b.instructions = out


def _fix_sync_waits(nc):
    """Post-schedule wait hygiene for walrus's per-struct sync-wait limits.

    1. Elide waits already implied by an earlier wait on the same engine
       (sem-ge is monotone and engines execute their instructions in order).
    2. For instructions still over their struct's wait capacity, INSERT
       NoOp wait-carriers on the same engine directly before them (strictly
       more conservative: the waits execute earlier in the same engine
       order).
    """
    import re

    _elidable = re.compile(r"^(DMASW|DMAHW|PE|DVE|Activation|Pool|SP)")
    # only instruction types whose sync_info round-trips cleanly may be
    # touched; anything else (raw-ISA customs, barriers, drains, branches)
    # is left intact and clears the elision state conservatively
    _touchable = (
        mybir.InstMatmult,
        mybir.InstNoOp,
        mybir.InstTensorTensor,
        mybir.InstTensorScalarPtr,
        mybir.InstActivation,
        mybir.InstTensorCopy,
        mybir.InstDMACopy,
        mybir.InstLdweights,
        mybir.InstMemset,
    )
    for f in nc.m.functions:
        for b in f.blocks:
            seen = {}
            for i in b.instructions:
                si = i.sync_info
                if si is None or not si.on_wait:
                    continue
                if not isinstance(i, _touchable):
                    seen.clear()
                    continue
                s = seen.setdefault(i.engine, {})
                kept = []
                for w in si.on_wait:
                    if (
                        w.wait_mode == "sem-ge-imm"
                        and _elidable.match(w.ant_name or "")
                        and s.get(w.id, -1) >= w.wait_value
                    ):
                        continue
                    kept.append(w)
                    if w.wait_mode == "sem-ge-imm" and _elidable.match(
                        w.ant_name or ""
                    ):
                        s[w.id] = w.wait_value
                if len(kept) != len(si.on_wait):
                    si.on_wait = kept

    # capacity per opcode (walrus setupSyncWait limits, found empirically:
    # Matmult fp32r=1, NoOp=1; others conservative)
    def cap_of(i):
        if isinstance(i, mybir.InstDrain):
            return 1  # spill the kernel-tail drain's wait pile onto NoOps
        if not isinstance(i, _touchable):
            return None
        return 1

    for f in nc.m.functions:
        for b in f.blocks:
            out = []
            for i in b.instructions:
                si = i.sync_info
                cap = cap_of(i)
                if si is not None and si.on_wait and cap is not None and len(
                    si.on_wait
                ) > cap:
                    waits = list(si.on_wait)
                    excess, keep = waits[:-cap], waits[-cap:]
                    while excess:
                        chunk, excess = excess[:1], excess[1:]
                        nop = mybir.InstNoOp(
                            name=nc.get_next_instruction_name(), ins=[], outs=[]
                        )
                        nop.engine = i.engine
                        nop.sync_info = mybir.SyncInfo(on_wait=chunk, on_update=[])
                        nop.bass_nofuse = True
                        out.append(nop)
                    si.on_wait = keep
                out.append(i)
            b.instructions = out


def build_program():
    global _PROGRAM
    if _PROGRAM is not None:
        return _PROGRAM
    nc = bass.Bass(trn_type="TRN2", target_bir_lowering=False, debug=False)
    xT = nc.dram_tensor("xT", [DIM, N], BF16, kind="ExternalInput").ap()
    wqT = nc.dram_tensor("wqT", [DIM, 256], BF16, kind="ExternalInput").ap()
    wkT = nc.dram_tensor("wkT", [DIM, 256], BF16, kind="ExternalInput").ap()
    wvT = nc.dram_tensor("wvT", [DIM, 256], BF16, kind="ExternalInput").ap()
    wo2T = nc.dram_tensor("wo2T", [128, 2, DIM], BF16, kind="ExternalInput").ap()
    expbT = nc.dram_tensor("expbT", [128, HL, 33, 128], BF16, kind="ExternalInput").ap()
    l_dram = nc.dram_tensor("l_scratch", [HL * 2, CW], F32, kind="Internal").ap()
    rec_dram = nc.dram_tensor("rec_scratch", [HL * 2, CW], F32, kind="Internal").ap()
    out_p = nc.dram_tensor("out_p", [N, DIM], BF16, kind="ExternalOutput").ap()

    with tile.TileContext(nc) as tc:
        _emit(tc, xT, wqT, wkT, wvT, wo2T, expbT, l_dram, rec_dram, out_p)
    _dedupe_ldweights(nc)
    _fix_sync_waits(nc)
    _PROGRAM = nc
    return nc


def make_in_maps(x, W_qkv, W_out, rel_emb):
    x = np.asarray(x, np.float32)
    W_qkv = np.asarray(W_qkv, np.float32)
    W_out = np.asarray(W_out, np.float32)
    rel_emb = np.asarray(rel_emb, np.float32)
    BF = ml_dtypes.bfloat16

    dd = np.arange(128)[:, None] - np.arange(128)[None, :]
    xTs = [np.ascontiguousarray(x[b].T).astype(BF) for b in range(x.shape[0])]
    woT = W_out.T  # [d, e]
    in_maps = []
    for c in range(8):
        b, g = c // 4, c % 4
        wq = W_qkv[g * 256 : (g + 1) * 256]
        wk = W_qkv[DIM + g * 256 : DIM + (g + 1) * 256] * np.float32(0.125)
        wv = W_qkv[2 * DIM + g * 256 : 2 * DIM + (g + 1) * 256]
        wo2 = np.ascontiguousarray(
            woT[256 * g : 256 * (g + 1)].reshape(2, 128, DIM).transpose(1, 0, 2)
        )
        # 33 Toeplitz tiles per head: e=8..24 are the mid window (block
        # delta kt-qi = 16-e), e<8 / e>24 saturate fully via the clip
        bT = np.empty((HL, 33, 128, 128), np.float32)
        for hl in range(HL):
            head = 4 * g + hl
            for e in range(33):
                i = e - 8
                idx = np.clip(128 * (8 - i) + dd, -1024, 1024) + 1024
                bT[hl, e] = np.exp(rel_emb[idx, head])
        in_maps.append(
            {
                "xT": xTs[b],
                "wqT": np.ascontiguousarray(wq.T).astype(BF),
                "wkT": np.ascontiguousarray(wk.T).astype(BF),
                "wvT": np.ascontiguousarray(wv.T).astype(BF),
                "wo2T": wo2.astype(BF),
                "expbT": np.ascontiguousarray(
                    bT.transpose(2, 0, 1, 3)
                ).astype(BF),
            }
        )
    return in_maps


def combine_outputs(results, b_out):
    b_out = np.asarray(b_out, np.float32)
    out = np.empty((2, N, DIM), np.float32)
    for b in range(2):
        acc = results[4 * b]["out_p"].astype(np.float32)
        for g in range(1, 4):
            acc = acc + results[4 * b + g]["out_p"].astype(np.float32)
        out[b] = acc + b_out[None, :]
    return out


def kernel(x, W_qkv, W_out, b_out, rel_emb):
    global LAST_RESULTS
    nc = build_program()
    in_maps = make_in_maps(x, W_qkv, W_out, rel_emb)
    LAST_RESULTS = run_bass_kernel_spmd(nc, in_maps, list(range(8)))
    return combine_outputs(LAST_RESULTS.results, b_out)
